# revision 18
# baseline (speedup 1.0000x reference)
"""Multi-head attention (B=4, N=2048, C=1024, H=16, D=64) on 8 TRN2 NeuronCores.

Sharding: core c handles batch b = c//2 and head-group g = c%2 (8 heads = 512
dims).  Each core computes qkv projection, attention, and a partial output
projection for its head slice; the host sums the two partials per batch and
adds the proj bias.

v2 layout (fp16 attention datapath, scalar-bound phase overlap):
  All matmul operands fp16 (PSUM accumulation stays f32).  Emission order:
  [x transposes + K for all slabs] -> [V all slabs] -> per slab: [Q(s);
  attention(s); normalize; transpose attn-out to d-major; proj(s)].  This
  starts the ScalarE exp stream (the hard per-chunk bottleneck: ~(N+352)/1.2
  ns per ACTIVATE) right after Q(0) instead of after the whole projection
  phase.
  S^T = k @ q^T per (slab, head-pair): two 64-contract matmuls in distinct
  PE row groups (fp16 weights -> separate LDWEIGHTS, streams overlap).
  PV is flipped: e (exp(S^T), fp16) is the stationary operand [128k x 128q],
  the ones-augmented V [128k x 65] streams; out[q, 0:64] = P@V rows and
  out[q, 64] = softmax denominator land in natural (q-major) layout, so the
  normalization is a per-partition tensor_scalar multiply with the DVE
  reciprocal of column 64.  Normalized slab output is PE-transposed back to
  d-major for the output projection.
  A fraction of exp chunks (DVE_CK) runs on the Vector engine instead of
  ScalarE via a 2-instruction Schraudolph approximation (t = A*s + B in f32,
  cast-to-int16, bitcast as fp16), unloading the scalar bottleneck.
"""

from contextlib import ExitStack

import numpy as np

import concourse.bass as bass
import concourse.tile as tile
from concourse import bacc, mybir
from concourse.bass_utils import run_bass_kernel_spmd
from concourse.masks import make_identity

P = 128
N = 2048          # tokens per batch
C = 1024          # model dim
DC = 512          # head dims per core (8 heads x 64)
NSLABS = N // 512
NCK = N // P      # 16 k-chunks
F32 = mybir.dt.float32
F32R = mybir.dt.float32r
F16 = mybir.dt.float16
I16 = mybir.dt.int16
ONE_F32_BITS = 0x3F800000

# Schraudolph exp on DVE: t = A*s_raw + B in f32, truncate to int16, bits
# reinterpreted as fp16 give ~exp(s_raw/8) (C=-44 minimax-centered under
# truncation, ~3% max rel err); the fp16 value is then upconverted to f32r.
# (A direct f32->i32 cast for an f32-bits variant fails neuronxcc lowering.)
SCHRAUD_A = (2.0 ** 10) * 1.4426950408889634 * 0.125
SCHRAUD_B = 15360.0 - 44.0
# k-chunks whose exp runs on the Vector engine (per (slab, pair) loop).
DVE_CK = (3, 9, 14)
# S-matmul scheme: 4x 64x64 row+col tiles (concurrent XBUS streams) vs
# 2x 64x128 row tiles (streams share one XBUS -> serialized).
S_QUAD = False


def build_program(trace_label: str = "attn"):
    nc = bacc.Bacc("TRN2", target_bir_lowering=False, name=trace_label)
    x_d = nc.dram_tensor("x", [N, C], F16, kind="ExternalInput").ap()
    wqkv_d = nc.dram_tensor("wqkv", [C, 3 * DC], F16, kind="ExternalInput").ap()
    wproj_d = nc.dram_tensor("wproj", [DC, C], F16, kind="ExternalInput").ap()
    out_d = nc.dram_tensor("out", [N, C], F32, kind="ExternalOutput").ap()

    with tile.TileContext(nc) as tc, ExitStack() as ctx:
        _emit(ctx, tc, x_d, wqkv_d, wproj_d, out_d)
    nc.compile()
    return nc


def _emit(ctx, tc, x_d, wqkv_d, wproj_d, out_d):
    nc = tc.nc

    const = ctx.enter_context(tc.tile_pool(name="const", bufs=1))
    ident32 = const.tile([P, P], F32, tag="ident32")
    make_identity(nc, ident32)
    ident = const.tile([P, P], F16, tag="ident")
    nc.vector.tensor_copy(ident[:], ident32[:])
    ones_row = const.tile([1, 64], F16, tag="ones_row")  # lhsT for denom bcast
    nc.any.memset(ones_row[:], 1.0)

    # Persistent SBUF tensors.  q/k d-major; v n-major, ones-augmented per
    # head (65th column = 1.0 so flipped PV emits the softmax denominator in
    # out[:, 64]); xt kept for all slabs (Q projection is deferred into the
    # per-slab attention region); aT is the d-major normalized attention out.
    persist = ctx.enter_context(tc.tile_pool(name="persist", bufs=1))
    qT = persist.tile([P, 4, N], F16, tag="qT")            # [d%128, d//128, n]
    kT = persist.tile([P, 4, N], F16, tag="kT")
    va = persist.tile([P, NCK, 8 * 65], F32R, tag="va")    # [n%128, n//128, 65h+dd]
    nc.any.memset(va[:].bitcast(mybir.dt.uint32), ONE_F32_BITS)
    xt = persist.tile([P, NSLABS, 8, 512], F16, tag="xt")  # [c%128, s, c//128, n]
    aT = persist.tile([P, 4, N], F16, tag="aT")
    wq = persist.tile([P, 8, 3 * DC], F16, tag="wq")       # [c%128, c//128, col]
    wp = persist.tile([P, 4, C], F16, tag="wp")

    # ---------------- region A: x^T, K and V for all slabs ----------------
    with tc.tile_pool(name="xnat", bufs=5) as xnat_pool, \
         tc.tile_pool(name="ps_tpA", bufs=2, space="PSUM") as ps_tpA, \
         tc.tile_pool(name="ps_warm", bufs=1, space="PSUM") as ps_warm, \
         tc.tile_pool(name="ps_kv", bufs=2, space="PSUM") as ps_kv:

        # PE warm-up while the first x/W DMAs land (HAM clock gate needs
        # ~3.4us of sustained matmul activity to reach 2.4 GHz).
        warm = ps_warm.tile([P, 512], F32, tag="warm")
        for _ in range(56):
            nc.tensor.matmul(warm[:, 0:P], ident[:], ident[:])

        for s in range(NSLABS):
            xn = []
            for i in range(4):
                t = xnat_pool.tile([P, C], F16, tag="xnat")
                r0 = s * 512 + i * P
                nc.sync.dma_start(t[:], x_d[r0:r0 + P, :])
                xn.append(t)
            if s == 0:
                for cc in range(8):
                    nc.scalar.dma_start(wq[:, cc, :], wqkv_d[cc * P:(cc + 1) * P, :])
                for dc in range(4):
                    nc.scalar.dma_start(wp[:, dc, :], wproj_d[dc * P:(dc + 1) * P, :])
            for cc in range(8):
                tp = ps_tpA.tile([P, 512], F16, tag="tpA")
                for i in range(4):
                    nc.tensor.transpose(
                        tp[:, i * P:(i + 1) * P],
                        xn[i][:, cc * P:(cc + 1) * P],
                        ident,
                    )
                nc.vector.tensor_copy(xt[:, s, cc, :], tp[:])
            # k^T for this slab (attention needs full kT before slab 0's S)
            for dc in range(4):
                ps = ps_kv.tile([P, 512], F32, tag="kv")
                col = DC + dc * P
                for cc in range(8):
                    nc.tensor.matmul(
                        ps[:],
                        wq[:, cc, col:col + P],
                        xt[:, s, cc, :],
                        start=(cc == 0),
                        stop=(cc == 7),
                    )
                nc.vector.tensor_copy(kT[:, dc, s * 512:(s + 1) * 512], ps[:])

    # ---------------- region B: per slab Q, attention, proj ----------------
    with tc.tile_pool(name="ps_st", bufs=2, space="PSUM") as ps_st, \
         tc.tile_pool(name="ps_pv", bufs=2, space="PSUM") as ps_pv, \
         tc.tile_pool(name="ps_mix", bufs=2, space="PSUM") as ps_mix, \
         tc.tile_pool(name="etile", bufs=4) as epool, \
         tc.tile_pool(name="ttile", bufs=1) as tpool, \
         tc.tile_pool(name="norm", bufs=4) as npool, \
         tc.tile_pool(name="oproj", bufs=2) as opool:

        def emit_proj(s):
            # output projection for slab s's n-chunks (deferred: emitted in
            # the middle of slab s+1's attention so the PE never stalls on
            # the DVE normalize chain of slab s)
            for i in range(4):
                nck = 4 * s + i
                for ct in range(2):
                    pp = ps_mix.tile([P, 512], F32, tag="mix",
                                     name=f"proj{nck}_{ct}")
                    for dc in range(4):
                        nc.tensor.matmul(
                            pp[:],
                            aT[:, dc, nck * P:(nck + 1) * P],
                            wp[:, dc, ct * 512:(ct + 1) * 512],
                            start=(dc == 0),
                            stop=(dc == 3),
                        )
                    ot = opool.tile([P, 512], F32, tag="ot")
                    nc.vector.tensor_copy(ot[:], pp[:])
                    nc.sync.dma_start(
                        out_d[nck * P:(nck + 1) * P, ct * 512:(ct + 1) * 512],
                        ot[:],
                    )

        def emit_v(j):
            # v for slab j (n-major, scattered into the 65-wide aug blocks);
            # emitted lazily inside slab 0's first pair so the exp stream
            # starts right after [transposes+K+Q(0)] instead of after V-all
            for i in range(4):
                ps = ps_mix.tile([P, 512], F32, tag="mix", name=f"v{j}_{i}")
                for cc in range(8):
                    nc.tensor.matmul(
                        ps[:],
                        xt[:, j, cc, i * P:(i + 1) * P],
                        wq[:, cc, 2 * DC:3 * DC],
                        start=(cc == 0),
                        stop=(cc == 7),
                    )
                for h in range(8):
                    nc.vector.tensor_copy(
                        va[:, 4 * j + i, 65 * h:65 * h + 64],
                        ps[:, 64 * h:64 * h + 64],
                    )

        def emit_normalize(s, pair, pvs):
            # aT[64sub+d, pair, q] = pv[d, q] / pv[64, q].  Copy P@V rows out
            # (casting to fp16), reciprocal the denominator row, PE-broadcast
            # it over the 64 head dims, scale in place.
            for sub in range(2):
                nc.vector.tensor_copy(
                    aT[64 * sub:64 * sub + 64, pair, s * 512:(s + 1) * 512],
                    pvs[sub][0:64, :],
                )
            for sub in range(2):
                dn = npool.tile([1, 512], F32, tag="dn",
                                name=f"dn_{pair}_{s}_{sub}")
                nc.vector.tensor_copy(dn[:], pvs[sub][64:65, :])
                rc32 = npool.tile([1, 512], F32, tag="rc32",
                                  name=f"rc32_{pair}_{s}_{sub}")
                nc.vector.reciprocal_approx_fast(rc32[:], dn[:])
                rc = npool.tile([1, 512], F16, tag="rc",
                                name=f"rc_{pair}_{s}_{sub}")
                nc.vector.tensor_copy(rc[:], rc32[:])
                bc = ps_mix.tile([P, 512], F32, tag="mix",
                                 name=f"bc_{pair}_{s}_{sub}")
                nc.tensor.matmul(bc[0:64, :], ones_row[:], rc[:])
                bcs = npool.tile([P, 512], F16, tag="bcs",
                                 name=f"bcs_{pair}_{s}_{sub}")
                o = 64 * sub
                nc.vector.tensor_copy(bcs[o:o + 64, :], bc[0:64, :])
                sl = aT[o:o + 64, pair, s * 512:(s + 1) * 512]
                nc.vector.tensor_mul(sl, sl, bcs[o:o + 64, :])

        deferred_norm = None
        for s in range(NSLABS):
            # Q for this slab (ps_mix shared by Q groups / V / bcasts / proj;
            # pool double-buffering serializes the overlaps)
            for dc in range(4):
                ps = ps_mix.tile([P, 512], F32, tag="mix", name=f"q{s}_{dc}")
                for cc in range(8):
                    nc.tensor.matmul(
                        ps[:],
                        wq[:, cc, dc * P:(dc + 1) * P],
                        xt[:, s, cc, :],
                        start=(cc == 0),
                        stop=(cc == 7),
                    )
                nc.vector.tensor_copy(qT[:, dc, s * 512:(s + 1) * 512], ps[:])
            # normalize of the previous slab's last pair was deferred past
            # Q(s) so Q never waits on the DVE normalize chain
            if deferred_norm is not None:
                emit_normalize(*deferred_norm)
                deferred_norm = None

            for pair in range(4):
                # two PV accumulators: [65 aug-d, 512 q] per head of the pair
                # (row 64 = softmax denominator via the ones-augmented va)
                pvs = [
                    ps_pv.tile([P, 512], F32, tag="pv", name=f"pv{pair}_{s}_{i}")
                    for i in range(2)
                ]

                def emit_pv(e_prev, ck_prev):
                    for sub in range(2):
                        h = 2 * pair + sub
                        nc.tensor.matmul(
                            pvs[sub][0:65, :],
                            va[:, ck_prev, 65 * h:65 * h + 65],
                            e_prev[:, sub, :],
                            start=(ck_prev == 0),
                            stop=(ck_prev == NCK - 1),
                        )

                # software pipeline depth 2: PV trails its exp by two chunks
                pending = []
                for ck in range(NCK):
                    if s == 0 and pair == 0 and ck in (0, 2, 6, 10):
                        emit_v((0, 1, 2, 3)[(0, 2, 6, 10).index(ck)])
                    st = ps_st.tile([P, 2, 512], F32, tag="st")
                    for sub in range(2):
                        o = 64 * sub
                        if S_QUAD:
                            # 4x 64x64 tiles: distinct col groups get their
                            # own XBUS -> the four streams run concurrently
                            for kh in range(2):
                                nc.tensor.matmul(
                                    st[64 * kh:64 * kh + 64, sub, :],
                                    kT[o:o + 64, pair,
                                       ck * P + 64 * kh:ck * P + 64 * kh + 64],
                                    qT[o:o + 64, pair, s * 512:(s + 1) * 512],
                                    tile_position=(o, 64 * kh),
                                    start=(kh == 0), stop=(kh == 1),
                                )
                        else:
                            nc.tensor.matmul(
                                st[:, sub, :],
                                kT[o:o + 64, pair, ck * P:(ck + 1) * P],
                                qT[o:o + 64, pair, s * 512:(s + 1) * 512],
                            )
                    e = epool.tile([P, 2, 512], F32R, tag="e")
                    if ck in DVE_CK:
                        t = tpool.tile([P, 2, 512], F32, tag="t")
                        nc.vector.tensor_scalar(
                            t[:], st[:], SCHRAUD_A, SCHRAUD_B,
                            mybir.AluOpType.mult, mybir.AluOpType.add,
                        )
                        eh = tpool.tile([P, 2, 512], F16, tag="eh")
                        nc.vector.tensor_copy(eh.bitcast(I16)[:], t[:])
                        nc.vector.tensor_copy(e[:], eh[:])
                    else:
                        nc.scalar.activation(
                            e[:], st[:], mybir.ActivationFunctionType.Exp,
                            scale=0.125,
                        )
                    pending.append((e, ck))
                    if len(pending) > 2:
                        emit_pv(*pending.pop(0))
                for item in pending:
                    emit_pv(*item)

                if pair == 3:
                    deferred_norm = (s, pair, pvs)
                else:
                    emit_normalize(s, pair, pvs)
                if pair == 0 and s > 0:
                    emit_proj(s - 1)
        emit_normalize(*deferred_norm)
        emit_proj(NSLABS - 1)


def shard_inputs(x, W_qkv, W_proj):
    """Full inputs -> 8 per-core in_maps. Core c: batch c//2, head-group c%2."""
    x = np.asarray(x, dtype=np.float32)
    W_qkv = np.asarray(W_qkv, dtype=np.float32)
    W_proj = np.asarray(W_proj, dtype=np.float32)
    in_maps = []
    for core in range(8):
        b, g = core // 2, core % 2
        cols = slice(g * DC, (g + 1) * DC)
        w = np.concatenate(
            [W_qkv[:, 0:C][:, cols], W_qkv[:, C:2 * C][:, cols],
             W_qkv[:, 2 * C:3 * C][:, cols]],
            axis=1,
        )
        in_maps.append({
            "x": np.ascontiguousarray(x[b]).astype(np.float16),
            "wqkv": np.ascontiguousarray(w).astype(np.float16),
            "wproj": np.ascontiguousarray(
                W_proj[g * DC:(g + 1) * DC, :]).astype(np.float16),
        })
    return in_maps


def unshard_output(results, b_proj):
    b_proj = np.asarray(b_proj, dtype=np.float32)
    out = np.empty((4, N, C), dtype=np.float32)
    for b in range(4):
        out[b] = results[2 * b]["out"] + results[2 * b + 1]["out"] + b_proj[None, :]
    return out


_NC_CACHE = []


def kernel(x, W_qkv, W_proj, b_proj, trace=False):
    in_maps = shard_inputs(x, W_qkv, W_proj)
    if not _NC_CACHE:
        _NC_CACHE.append(build_program())
    nc = _NC_CACHE[0]
    res = run_bass_kernel_spmd(nc, in_maps, core_ids=list(range(8)), trace=trace)
    out = unshard_output(res.results, b_proj)
    if trace:
        return out, res
    return out


# revision 19
# speedup vs baseline: 1.2342x; 1.2342x over previous
"""Multi-head attention (B=4, N=2048, C=1024, H=16, D=64) on 8 TRN2 NeuronCores.

Sharding: core c handles batch b = c//2 and head-group g = c%2 (8 heads = 512
dims).  Each core computes qkv projection, attention, and a partial output
projection for its head slice; the host sums the two partials per batch and
adds the proj bias.

v2 layout (fp16 attention datapath, scalar-bound phase overlap):
  All matmul operands fp16 (PSUM accumulation stays f32).  Emission order:
  [x transposes + K for all slabs] -> [V all slabs] -> per slab: [Q(s);
  attention(s); normalize; transpose attn-out to d-major; proj(s)].  This
  starts the ScalarE exp stream (the hard per-chunk bottleneck: ~(N+352)/1.2
  ns per ACTIVATE) right after Q(0) instead of after the whole projection
  phase.
  S^T = k @ q^T per (slab, head-pair): two 64-contract matmuls in distinct
  PE row groups (fp16 weights -> separate LDWEIGHTS, streams overlap).
  PV is flipped: e (exp(S^T), fp16) is the stationary operand [128k x 128q],
  the ones-augmented V [128k x 65] streams; out[q, 0:64] = P@V rows and
  out[q, 64] = softmax denominator land in natural (q-major) layout, so the
  normalization is a per-partition tensor_scalar multiply with the DVE
  reciprocal of column 64.  Normalized slab output is PE-transposed back to
  d-major for the output projection.
  A fraction of exp chunks (DVE_CK) runs on the Vector engine instead of
  ScalarE via a 2-instruction Schraudolph approximation (t = A*s + B in f32,
  cast-to-int16, bitcast as fp16), unloading the scalar bottleneck.
"""

from contextlib import ExitStack

import numpy as np

import concourse.bass as bass
import concourse.tile as tile
from concourse import bacc, mybir
from concourse.bass_utils import run_bass_kernel_spmd
from concourse.masks import make_identity

P = 128
N = 2048          # tokens per batch
C = 1024          # model dim
DC = 512          # head dims per core (8 heads x 64)
NSLABS = N // 512
NCK = N // P      # 16 k-chunks
F32 = mybir.dt.float32
F32R = mybir.dt.float32r
F16 = mybir.dt.float16
I16 = mybir.dt.int16
ONE_F32_BITS = 0x3F800000

# Schraudolph exp on DVE: t = A*s_raw + B in f32, truncate to int16, bits
# reinterpreted as fp16 give ~exp(s_raw/8) (C=-44 minimax-centered under
# truncation, ~3% max rel err); the fp16 value is then upconverted to f32r.
# (A direct f32->i32 cast for an f32-bits variant fails neuronxcc lowering.)
SCHRAUD_A = (2.0 ** 10) * 1.4426950408889634 * 0.125
SCHRAUD_B = 15360.0 - 44.0
# k-chunks whose exp runs on the Vector engine (per (slab, pair) loop).
DVE_CK = ()
# S-matmul scheme: 4x 64x64 row+col tiles (concurrent XBUS streams) vs
# 2x 64x128 row tiles (streams share one XBUS -> serialized).
S_QUAD = False


def build_program(trace_label: str = "attn"):
    nc = bacc.Bacc("TRN2", target_bir_lowering=False, name=trace_label)
    x_d = nc.dram_tensor("x", [N, C], F16, kind="ExternalInput").ap()
    wqkv_d = nc.dram_tensor("wqkv", [C, 3 * DC], F16, kind="ExternalInput").ap()
    wproj_d = nc.dram_tensor("wproj", [DC, C], F16, kind="ExternalInput").ap()
    out_d = nc.dram_tensor("out", [N, C], F32, kind="ExternalOutput").ap()

    with tile.TileContext(nc) as tc, ExitStack() as ctx:
        _emit(ctx, tc, x_d, wqkv_d, wproj_d, out_d)
    nc.compile()
    return nc


def _emit(ctx, tc, x_d, wqkv_d, wproj_d, out_d):
    nc = tc.nc

    const = ctx.enter_context(tc.tile_pool(name="const", bufs=1))
    ident32 = const.tile([P, P], F32, tag="ident32")
    make_identity(nc, ident32)
    ident = const.tile([P, P], F16, tag="ident")
    nc.vector.tensor_copy(ident[:], ident32[:])
    ones_row = const.tile([1, 64], F16, tag="ones_row")  # lhsT for denom bcast
    nc.any.memset(ones_row[:], 1.0)

    # Persistent SBUF tensors.  q/k d-major; v n-major, ones-augmented per
    # head (65th column = 1.0 so flipped PV emits the softmax denominator in
    # out[:, 64]); xt kept for all slabs (Q projection is deferred into the
    # per-slab attention region); aT is the d-major normalized attention out.
    persist = ctx.enter_context(tc.tile_pool(name="persist", bufs=1))
    qT = persist.tile([P, 4, N], F16, tag="qT")            # [d%128, d//128, n]
    kT = persist.tile([P, 4, N], F16, tag="kT")
    va = persist.tile([P, NCK, 8 * 65], F32R, tag="va")    # [n%128, n//128, 65h+dd]
    nc.any.memset(va[:].bitcast(mybir.dt.uint32), ONE_F32_BITS)
    xt = persist.tile([P, NSLABS, 8, 512], F16, tag="xt")  # [c%128, s, c//128, n]
    aT = persist.tile([P, 4, N], F16, tag="aT")
    wq = persist.tile([P, 8, 3 * DC], F16, tag="wq")       # [c%128, c//128, col]
    wp = persist.tile([P, 4, C], F16, tag="wp")

    # ---------------- region A: x^T, K and V for all slabs ----------------
    with tc.tile_pool(name="xnat", bufs=5) as xnat_pool, \
         tc.tile_pool(name="ps_tpA", bufs=2, space="PSUM") as ps_tpA, \
         tc.tile_pool(name="ps_warm", bufs=1, space="PSUM") as ps_warm, \
         tc.tile_pool(name="ps_kv", bufs=2, space="PSUM") as ps_kv:

        # PE warm-up while the first x/W DMAs land (HAM clock gate needs
        # ~3.4us of sustained matmul activity to reach 2.4 GHz).
        warm = ps_warm.tile([P, 512], F32, tag="warm")
        for _ in range(56):
            nc.tensor.matmul(warm[:, 0:P], ident[:], ident[:])

        for s in range(NSLABS):
            xn = []
            for i in range(4):
                t = xnat_pool.tile([P, C], F16, tag="xnat")
                r0 = s * 512 + i * P
                nc.sync.dma_start(t[:], x_d[r0:r0 + P, :])
                xn.append(t)
            if s == 0:
                for cc in range(8):
                    nc.scalar.dma_start(wq[:, cc, :], wqkv_d[cc * P:(cc + 1) * P, :])
                for dc in range(4):
                    nc.scalar.dma_start(wp[:, dc, :], wproj_d[dc * P:(dc + 1) * P, :])
            for cc in range(8):
                tp = ps_tpA.tile([P, 512], F16, tag="tpA")
                for i in range(4):
                    nc.tensor.transpose(
                        tp[:, i * P:(i + 1) * P],
                        xn[i][:, cc * P:(cc + 1) * P],
                        ident,
                    )
                nc.vector.tensor_copy(xt[:, s, cc, :], tp[:])
            # k^T for this slab (attention needs full kT before slab 0's S)
            for dc in range(4):
                ps = ps_kv.tile([P, 512], F32, tag="kv")
                col = DC + dc * P
                for cc in range(8):
                    nc.tensor.matmul(
                        ps[:],
                        wq[:, cc, col:col + P],
                        xt[:, s, cc, :],
                        start=(cc == 0),
                        stop=(cc == 7),
                    )
                nc.vector.tensor_copy(kT[:, dc, s * 512:(s + 1) * 512], ps[:])

    # ---------------- region B: per slab Q, attention, proj ----------------
    with tc.tile_pool(name="ps_st", bufs=2, space="PSUM") as ps_st, \
         tc.tile_pool(name="ps_pv", bufs=2, space="PSUM") as ps_pv, \
         tc.tile_pool(name="ps_mix", bufs=2, space="PSUM") as ps_mix, \
         tc.tile_pool(name="etile", bufs=4) as epool, \
         tc.tile_pool(name="ttile", bufs=1) as tpool, \
         tc.tile_pool(name="norm", bufs=4) as npool, \
         tc.tile_pool(name="oproj", bufs=2) as opool:

        def emit_proj(s):
            # output projection for slab s's n-chunks (deferred: emitted in
            # the middle of slab s+1's attention so the PE never stalls on
            # the DVE normalize chain of slab s)
            for i in range(4):
                nck = 4 * s + i
                for ct in range(2):
                    pp = ps_mix.tile([P, 512], F32, tag="mix",
                                     name=f"proj{nck}_{ct}")
                    for dc in range(4):
                        nc.tensor.matmul(
                            pp[:],
                            aT[:, dc, nck * P:(nck + 1) * P],
                            wp[:, dc, ct * 512:(ct + 1) * 512],
                            start=(dc == 0),
                            stop=(dc == 3),
                        )
                    ot = opool.tile([P, 512], F32, tag="ot")
                    nc.vector.tensor_copy(ot[:], pp[:])
                    nc.sync.dma_start(
                        out_d[nck * P:(nck + 1) * P, ct * 512:(ct + 1) * 512],
                        ot[:],
                    )

        def emit_v(j):
            # v for slab j (n-major, scattered into the 65-wide aug blocks);
            # emitted lazily inside slab 0's first pair so the exp stream
            # starts right after [transposes+K+Q(0)] instead of after V-all
            for i in range(4):
                ps = ps_mix.tile([P, 512], F32, tag="mix", name=f"v{j}_{i}")
                for cc in range(8):
                    nc.tensor.matmul(
                        ps[:],
                        xt[:, j, cc, i * P:(i + 1) * P],
                        wq[:, cc, 2 * DC:3 * DC],
                        start=(cc == 0),
                        stop=(cc == 7),
                    )
                for h in range(8):
                    nc.vector.tensor_copy(
                        va[:, 4 * j + i, 65 * h:65 * h + 64],
                        ps[:, 64 * h:64 * h + 64],
                    )

        def emit_normalize(s, pair, pvs):
            # aT[64sub+d, pair, q] = pv[d, q] / pv[64, q].  Copy P@V rows out
            # (casting to fp16), reciprocal the denominator row, PE-broadcast
            # it over the 64 head dims, scale in place.
            for sub in range(2):
                nc.vector.tensor_copy(
                    aT[64 * sub:64 * sub + 64, pair, s * 512:(s + 1) * 512],
                    pvs[sub][0:64, :],
                )
            for sub in range(2):
                dn = npool.tile([1, 512], F32, tag="dn",
                                name=f"dn_{pair}_{s}_{sub}")
                nc.vector.tensor_copy(dn[:], pvs[sub][64:65, :])
                rc32 = npool.tile([1, 512], F32, tag="rc32",
                                  name=f"rc32_{pair}_{s}_{sub}")
                nc.vector.reciprocal_approx_fast(rc32[:], dn[:])
                rc = npool.tile([1, 512], F16, tag="rc",
                                name=f"rc_{pair}_{s}_{sub}")
                nc.vector.tensor_copy(rc[:], rc32[:])
                bc = ps_mix.tile([P, 512], F32, tag="mix",
                                 name=f"bc_{pair}_{s}_{sub}")
                nc.tensor.matmul(bc[0:64, :], ones_row[:], rc[:])
                bcs = npool.tile([P, 512], F16, tag="bcs",
                                 name=f"bcs_{pair}_{s}_{sub}")
                o = 64 * sub
                nc.vector.tensor_copy(bcs[o:o + 64, :], bc[0:64, :])
                sl = aT[o:o + 64, pair, s * 512:(s + 1) * 512]
                nc.vector.tensor_mul(sl, sl, bcs[o:o + 64, :])

        deferred_norm = None
        for s in range(NSLABS):
            # Q for this slab (ps_mix shared by Q groups / V / bcasts / proj;
            # pool double-buffering serializes the overlaps)
            for dc in range(4):
                ps = ps_mix.tile([P, 512], F32, tag="mix", name=f"q{s}_{dc}")
                for cc in range(8):
                    nc.tensor.matmul(
                        ps[:],
                        wq[:, cc, dc * P:(dc + 1) * P],
                        xt[:, s, cc, :],
                        start=(cc == 0),
                        stop=(cc == 7),
                    )
                nc.vector.tensor_copy(qT[:, dc, s * 512:(s + 1) * 512], ps[:])
            # normalize of the previous slab's last pair was deferred past
            # Q(s) so Q never waits on the DVE normalize chain
            if deferred_norm is not None:
                emit_normalize(*deferred_norm)
                deferred_norm = None

            for pair in range(4):
                # two PV accumulators: [65 aug-d, 512 q] per head of the pair
                # (row 64 = softmax denominator via the ones-augmented va)
                pvs = [
                    ps_pv.tile([P, 512], F32, tag="pv", name=f"pv{pair}_{s}_{i}")
                    for i in range(2)
                ]

                def emit_pv(e_prev, ck_prev):
                    for sub in range(2):
                        h = 2 * pair + sub
                        nc.tensor.matmul(
                            pvs[sub][0:65, :],
                            va[:, ck_prev, 65 * h:65 * h + 65],
                            e_prev[:, sub, :],
                            start=(ck_prev == 0),
                            stop=(ck_prev == NCK - 1),
                        )

                # software pipeline depth 2: PV trails its exp by two chunks
                pending = []
                for ck in range(NCK):
                    if s == 0 and pair == 0 and ck in (0, 2, 6, 10):
                        emit_v((0, 1, 2, 3)[(0, 2, 6, 10).index(ck)])
                    st = ps_st.tile([P, 2, 512], F32, tag="st")
                    for sub in range(2):
                        o = 64 * sub
                        if S_QUAD:
                            # 4x 64x64 tiles: distinct col groups get their
                            # own XBUS -> the four streams run concurrently
                            for kh in range(2):
                                nc.tensor.matmul(
                                    st[64 * kh:64 * kh + 64, sub, :],
                                    kT[o:o + 64, pair,
                                       ck * P + 64 * kh:ck * P + 64 * kh + 64],
                                    qT[o:o + 64, pair, s * 512:(s + 1) * 512],
                                    tile_position=(o, 64 * kh),
                                    start=(kh == 0), stop=(kh == 1),
                                )
                        else:
                            nc.tensor.matmul(
                                st[:, sub, :],
                                kT[o:o + 64, pair, ck * P:(ck + 1) * P],
                                qT[o:o + 64, pair, s * 512:(s + 1) * 512],
                            )
                    e = epool.tile([P, 2, 512], F32R, tag="e")
                    if ck in DVE_CK:
                        t = tpool.tile([P, 2, 512], F32, tag="t")
                        nc.vector.tensor_scalar(
                            t[:], st[:], SCHRAUD_A, SCHRAUD_B,
                            mybir.AluOpType.mult, mybir.AluOpType.add,
                        )
                        eh = tpool.tile([P, 2, 512], F16, tag="eh")
                        nc.vector.tensor_copy(eh.bitcast(I16)[:], t[:])
                        nc.vector.tensor_copy(e[:], eh[:])
                    else:
                        nc.scalar.activation(
                            e[:], st[:], mybir.ActivationFunctionType.Exp,
                            scale=0.125,
                        )
                    pending.append((e, ck))
                    if len(pending) > 2:
                        emit_pv(*pending.pop(0))
                for item in pending:
                    emit_pv(*item)

                if pair == 3:
                    deferred_norm = (s, pair, pvs)
                else:
                    emit_normalize(s, pair, pvs)
                if pair == 0 and s > 0:
                    emit_proj(s - 1)
        emit_normalize(*deferred_norm)
        emit_proj(NSLABS - 1)


def shard_inputs(x, W_qkv, W_proj):
    """Full inputs -> 8 per-core in_maps. Core c: batch c//2, head-group c%2."""
    x = np.asarray(x, dtype=np.float32)
    W_qkv = np.asarray(W_qkv, dtype=np.float32)
    W_proj = np.asarray(W_proj, dtype=np.float32)
    in_maps = []
    for core in range(8):
        b, g = core // 2, core % 2
        cols = slice(g * DC, (g + 1) * DC)
        w = np.concatenate(
            [W_qkv[:, 0:C][:, cols], W_qkv[:, C:2 * C][:, cols],
             W_qkv[:, 2 * C:3 * C][:, cols]],
            axis=1,
        )
        in_maps.append({
            "x": np.ascontiguousarray(x[b]).astype(np.float16),
            "wqkv": np.ascontiguousarray(w).astype(np.float16),
            "wproj": np.ascontiguousarray(
                W_proj[g * DC:(g + 1) * DC, :]).astype(np.float16),
        })
    return in_maps


def unshard_output(results, b_proj):
    b_proj = np.asarray(b_proj, dtype=np.float32)
    out = np.empty((4, N, C), dtype=np.float32)
    for b in range(4):
        out[b] = results[2 * b]["out"] + results[2 * b + 1]["out"] + b_proj[None, :]
    return out


_NC_CACHE = []


def kernel(x, W_qkv, W_proj, b_proj, trace=False):
    in_maps = shard_inputs(x, W_qkv, W_proj)
    if not _NC_CACHE:
        _NC_CACHE.append(build_program())
    nc = _NC_CACHE[0]
    res = run_bass_kernel_spmd(nc, in_maps, core_ids=list(range(8)), trace=trace)
    out = unshard_output(res.results, b_proj)
    if trace:
        return out, res
    return out
